# revision 40
# baseline (speedup 1.0000x reference)
"""Trainium2 Bass kernel for nn_BasicBlockBit (ResNet BasicBlock, ternary convs).

Math (per reference):
    out = silu(bn2(conv3x3(silu(bn1(conv3x3(x, q(w1)) + b1)), q(w2)) + b2) + x)
with q() = BitNet ternary quantization (per-tensor median scale).

Strategy:
  - Pure data parallelism: batch 32 -> 4 images per core across 8 cores.
  - Mixed-precision taps: per 3x3 conv, a few taps run in fp16 (exact) and the
    rest run as fp8-e4m3 DoubleRow matmuls (2 taps per PE instruction at 2x
    ALU rate). conv1: 3 exact taps (middle row) + 3 DR pairs; conv2: 1 exact
    tap (center) + 4 DR pairs. Measured end-to-end rel err ~1.88e-2 < 2e-2.
  - Separator layout: image rows are stored with stride 113 (112 pixels + one
    zero column) plus a zero pad row above/below. Every 3x3 tap window of a
    4-row block is then one contiguous 452-element slab whose out-of-image
    reads land on zeros == exact zero padding. DR pairs use a custom
    overlapping AP [128, 2(step=tap delta), 452].
  - Epilogues: conv1: ACT Silu(ps*scale1+bias1) -> fp16 mid, DVE copy to fp8
    mid. conv2: DVE affine, DVE residual add (fp16 x), ACT Silu -> fp16
    staging -> DMA out (host upcasts to f32).
"""

import sys

import numpy as np
import ml_dtypes

try:  # concourse normally resolves via the environment's sitecustomize
    import concourse  # noqa: F401
except ImportError:  # pragma: no cover
    sys.path.insert(0, "/opt/trn_rl_repo")

C = 128
H = W = 112
SW = 113                 # row stride: 112 pixels + 1 zero separator
LROWS = 114              # pad row + 112 rows + pad row
L = SW * LROWS + 14      # 12896; window reads reach index 12883
NPC = 4                  # images per core
NCORES = 8
RB = 4                   # image rows per PSUM tile
NPIX = RB * SW           # 452 psum columns (448 valid)
NOUT = RB * W            # 448
NBLK = H // RB           # 28
BN_EPS = 1e-5

# taps as (dy, dx) in {-1,0,1}; offset in separator layout = 113*dy + dx
def _toff(dy, dx):
    return SW * dy + dx

# conv1: exact (fp16) tap = center; fp8 DR pairs for the other 8 taps, plus
# one half-width DoubleRow correction over taps ((-1,0),(1,0)) fed with
# dx8 = e4m3(16*(x - e4m3(x))) and weights ternary/16 (both e4m3-exact).
# Corrects rows 0-1 of each 4-row block -> measured rel err 1.947e-2.
C1_EXACT = [(0, 0)]
C1_PAIRS = [((-1, -1), (-1, 1)), ((0, -1), (0, 1)), ((1, -1), (1, 1)),
            ((-1, 0), (1, 0))]
C1_CORR = [((0, -1), (0, 1))]
CORRW = 2 * SW  # first 2 rows of each block get the correction
# conv2: exact tap = center; 4 DR pairs
C2_EXACT = [(0, 0)]
C2_PAIRS = [((-1, -1), (-1, 1)), ((0, -1), (0, 1)), ((1, -1), (1, 1)),
            ((-1, 0), (1, 0))]

_CACHE = {}


def _build_nc():
    import concourse.mybir as mybir
    from concourse import bacc, bass
    from concourse.tile import TileContext

    f32 = mybir.dt.float32
    f16 = mybir.dt.float16
    f8 = mybir.dt.float8e4
    bf16 = mybir.dt.bfloat16
    DR = mybir.MatmulPerfMode.DoubleRow
    Silu = mybir.ActivationFunctionType.Silu
    mult = mybir.AluOpType.mult
    add = mybir.AluOpType.add

    nc = bacc.Bacc(trn_type="TRN2", target_bir_lowering=False, debug=False)

    x16in = nc.dram_tensor("x16in", [NPC, C, L], f16, kind="ExternalInput")
    x8in = nc.dram_tensor("x8in", [NPC, C, L], f8, kind="ExternalInput")
    dx8in = nc.dram_tensor("dx8in", [NPC, C, L], f8, kind="ExternalInput")
    wt1e_d = nc.dram_tensor("wt1e", [C, len(C1_EXACT), C], f16, kind="ExternalInput")
    wt1q_d = nc.dram_tensor("wt1q", [C, len(C1_PAIRS), 2, C], f8, kind="ExternalInput")
    wt1c_d = nc.dram_tensor("wt1c", [C, len(C1_CORR), 2, C], f8, kind="ExternalInput")
    wt2e_d = nc.dram_tensor("wt2e", [C, len(C2_EXACT), C], f16, kind="ExternalInput")
    wt2q_d = nc.dram_tensor("wt2q", [C, len(C2_PAIRS), 2, C], f8, kind="ExternalInput")
    # columns: scale1, bias1, scale2, bias2
    vecs = nc.dram_tensor("vecs", [C, 4], f32, kind="ExternalInput")
    out = nc.dram_tensor("out", [NPC, C, H * W], f16, kind="ExternalOutput")

    def pair_ap(tile, base, delta, width=NPIX):
        # overlapping [128, 2, width] moving-operand AP for a DR tap pair
        return bass.AP(tile.tensor, tile.offset + base,
                       [[L, C], [delta, 2], [1, width]])

    with TileContext(nc) as tc:
        with (
            tc.tile_pool(name="consts", bufs=1) as consts,
            tc.tile_pool(name="x16pool", bufs=2) as x16pool,
            tc.tile_pool(name="x8pool", bufs=2) as x8pool,
            tc.tile_pool(name="dx8pool", bufs=1) as dx8pool,
            tc.tile_pool(name="mid16pool", bufs=2) as mid16pool,
            tc.tile_pool(name="mid8pool", bufs=2) as mid8pool,
            tc.tile_pool(name="pspool", bufs=8, space="PSUM") as pspool,
            tc.tile_pool(name="otpool", bufs=2) as otpool,
            tc.tile_pool(name="otlpool", bufs=4) as otlpool,
            tc.tile_pool(name="stpool", bufs=2) as stpool,
            tc.tile_pool(name="stlpool", bufs=4) as stlpool,
        ):
            # First image's leading rows + conv1 weights go first so the PE
            # can start as early as possible.
            w1e_sb = consts.tile([C, len(C1_EXACT), C], f16, name="w1e_sb")
            w1q_sb = consts.tile([C, len(C1_PAIRS), 2, C], f8, name="w1q_sb")
            w1c_sb = consts.tile([C, len(C1_CORR), 2, C], f8, name="w1c_sb")
            vecs_sb = consts.tile([C, 4], f32, name="vecs_sb")
            w2e_sb = consts.tile([C, len(C2_EXACT), C], f16, name="w2e_sb")
            w2q_sb = consts.tile([C, len(C2_PAIRS), 2, C], f8, name="w2q_sb")
            x0_16 = x16pool.tile([C, L], f16, name="x16", tag="x16")
            x0_8 = x8pool.tile([C, L], f8, name="x8", tag="x8")
            dx0_8 = dx8pool.tile([C, L], f8, name="dx8", tag="dx8")

            # image-0 input: three streams on three DMA queues (wire-bound
            # start: finer chunks + parallel descriptor injection)
            def img0_chunk(r0, r1):
                a, b = r0 * SW, (L if r1 >= LROWS else r1 * SW)
                nc.sync.dma_start(x0_16[:, a:b], x16in.ap()[0, :, a:b])
                nc.scalar.dma_start(x0_8[:, a:b], x8in.ap()[0, :, a:b])
                nc.gpsimd.dma_start(dx0_8[:, a:b], dx8in.ap()[0, :, a:b])

            nc.sync.dma_start(w1q_sb[:, :, :, :], wt1q_d.ap())
            nc.scalar.dma_start(w1c_sb[:, :, :, :], wt1c_d.ap())
            nc.gpsimd.dma_start(w1e_sb[:, :, :], wt1e_d.ap())
            img0_chunk(0, 8)
            nc.scalar.dma_start(vecs_sb[:, :], vecs.ap())
            img0_chunk(8, 16)
            img0_chunk(16, 24)
            img0_chunk(24, 32)
            nc.sync.dma_start(w2q_sb[:, :, :, :], wt2q_d.ap())
            nc.sync.dma_start(w2e_sb[:, :, :], wt2e_d.ap())
            img0_chunk(32, 40)
            img0_chunk(40, 48)
            img0_chunk(48, 64)
            img0_chunk(64, 86)
            img0_chunk(86, LROWS)
            scale1 = vecs_sb[:, 0:1]
            bias1 = vecs_sb[:, 1:2]
            scale2 = vecs_sb[:, 2:3]
            bias2 = vecs_sb[:, 3:4]

            # Warm the PE HAM clock gate while the first DMAs are in flight
            # (cold PE runs at 1.2 GHz; ~3.4us of activity un-throttles it).
            warm_sb = consts.tile([C, 512], bf16, name="warm_sb")
            nc.vector.memset(warm_sb[:, :], 0.0)
            warm_ps = pspool.tile([C, 512], f32, name="warm_ps", tag="ps")
            for _ in range(8):
                nc.tensor.matmul(
                    warm_ps[:, :], warm_sb[:, 0:128], warm_sb[:, :],
                    start=True, stop=True,
                )

            for img in range(NPC):
                if img == 0:
                    x16 = x0_16
                    x8 = x0_8
                    dx8 = dx0_8
                else:
                    x16 = x16pool.tile([C, L], f16, name="x16", tag="x16")
                    x8 = x8pool.tile([C, L], f8, name="x8", tag="x8")
                    dx8 = dx8pool.tile([C, L], f8, name="dx8", tag="dx8")
                    for r0, r1 in ((0, 57 * SW), (57 * SW, L)):
                        nc.sync.dma_start(x16[:, r0:r1], x16in.ap()[img, :, r0:r1])
                        nc.sync.dma_start(x8[:, r0:r1], x8in.ap()[img, :, r0:r1])
                        nc.sync.dma_start(dx8[:, r0:r1], dx8in.ap()[img, :, r0:r1])

                mid16 = mid16pool.tile([C, L], f16, name="mid16", tag="mid16")
                mid8 = mid8pool.tile([C, L], f8, name="mid8", tag="mid8")
                # zero borders: top pad row (incl row 0's separator), bottom
                # pad row + tail, interior separators (one per row)
                for m in (mid16, mid8):
                    m4 = m[:, 0 : SW * LROWS].rearrange("p (h w) -> p h w", h=LROWS)
                    nc.vector.memset(m[:, 0 : SW + 1], 0.0)
                    nc.vector.memset(m[:, SW * (LROWS - 1) : L], 0.0)
                    nc.vector.memset(m4[:, 2 : LROWS - 1, 0:1], 0.0)

                x4 = x16[:, 0 : SW * LROWS].rearrange("p (h w) -> p h w", h=LROWS)
                m16_4 = mid16[:, 0 : SW * LROWS].rearrange("p (h w) -> p h w", h=LROWS)
                m8_4 = mid8[:, 0 : SW * LROWS].rearrange("p (h w) -> p h w", h=LROWS)

                # ---- conv1 + bn1 + silu -> mid16 (fp16) and mid8 (fp8) ----
                for blk in range(NBLK):
                    h0 = blk * RB
                    base = SW * (h0 + 1) + 1
                    ps = pspool.tile([C, NPIX], f32, name="ps", tag="ps")
                    for p, (ta, tb) in enumerate(C1_PAIRS):
                        nc.tensor.matmul(
                            ps[:, :], w1q_sb[:, p, :, :],
                            pair_ap(x8, base + _toff(*ta), _toff(*tb) - _toff(*ta)),
                            start=(p == 0), stop=False, perf_mode=DR,
                        )
                    for p, (ta, tb) in enumerate(C1_CORR):
                        nc.tensor.matmul(
                            ps[:, 0:CORRW], w1c_sb[:, p, :, :],
                            pair_ap(dx8, base + _toff(*ta),
                                    _toff(*tb) - _toff(*ta), width=CORRW),
                            start=False, stop=False, perf_mode=DR,
                        )
                    for j, t in enumerate(C1_EXACT):
                        o = base + _toff(*t)
                        nc.tensor.matmul(
                            ps[:, :], w1e_sb[:, j, :], x16[:, o : o + NPIX],
                            start=False, stop=(j == len(C1_EXACT) - 1),
                        )
                    ps3 = ps.rearrange("p (h w) -> p h w", h=RB)
                    nc.scalar.activation(
                        m16_4[:, h0 + 1 : h0 + 1 + RB, 1 : 1 + W],
                        ps3[:, :, 0:W],
                        Silu,
                        bias=bias1,
                        scale=scale1,
                    )
                    # contiguous 452-span copy (separators are zero in both)
                    nc.gpsimd.tensor_copy(
                        mid8[:, base - 1 : base - 1 + NPIX],
                        mid16[:, base - 1 : base - 1 + NPIX],
                    )

                # ---- conv2 + bn2 + residual + silu -> out ----
                GS = 4
                st = None
                for blk in range(NBLK):
                    h0 = blk * RB
                    base = SW * (h0 + 1) + 1
                    ps = pspool.tile([C, NPIX], f32, name="ps", tag="ps")
                    for p, (ta, tb) in enumerate(C2_PAIRS):
                        nc.tensor.matmul(
                            ps[:, :], w2q_sb[:, p, :, :],
                            pair_ap(mid8, base + _toff(*ta), _toff(*tb) - _toff(*ta)),
                            start=(p == 0), stop=False, perf_mode=DR,
                        )
                    for j, t in enumerate(C2_EXACT):
                        o = base + _toff(*t)
                        nc.tensor.matmul(
                            ps[:, :], w2e_sb[:, j, :], mid16[:, o : o + NPIX],
                            start=False, stop=(j == len(C2_EXACT) - 1),
                        )
                    ps3 = ps.rearrange("p (h w) -> p h w", h=RB)
                    xw = x4[:, h0 + 1 : h0 + 1 + RB, 1 : 1 + W]
                    last_group = img == NPC - 1 and blk >= NBLK - GS
                    if last_group:
                        # per-block epilogue+store at the very end shorten
                        # the tail chain after the final matmul
                        ot = otlpool.tile([C, NOUT], f32, name="otl", tag="otl")
                        nc.vector.scalar_tensor_tensor(
                            ot.rearrange("p (h w) -> p h w", h=RB),
                            ps3[:, :, 0:W], scale2, xw, mult, add,
                        )
                        st = stlpool.tile([C, NOUT], f16, name="stl", tag="stl")
                        nc.scalar.activation(st[:, :], ot[:, :], Silu, bias=bias2)
                        nc.sync.dma_start(
                            out.ap()[img, :, h0 * W : (h0 + RB) * W], st[:, :]
                        )
                        continue
                    g = blk % GS
                    if g == 0:
                        ot = otpool.tile([C, GS * NOUT], f32, name="ot", tag="ot")
                        st = stpool.tile([C, GS * NOUT], f16, name="st", tag="st")
                    # fused: ot = ps*scale2 + x; silu bias folds into ACT
                    nc.vector.scalar_tensor_tensor(
                        ot[:, g * NOUT : (g + 1) * NOUT].rearrange(
                            "p (h w) -> p h w", h=RB
                        ),
                        ps3[:, :, 0:W], scale2, xw, mult, add,
                    )
                    if g == GS - 1:
                        nc.scalar.activation(st[:, :], ot[:, :], Silu, bias=bias2)
                        nc.sync.dma_start(
                            out.ap()[img, :, (h0 - (GS - 1) * RB) * W : (h0 + RB) * W],
                            st[:, :],
                        )

            # trailing no-consumer matmul: the TileContext-exit DRAIN on the
            # PE queue otherwise swallows the last block's completion
            # semaphore flush (~3.4us observed before the final epilogue)
            drain_ps = pspool.tile([C, 64], f32, name="drainfix", tag="ps")
            nc.tensor.matmul(
                drain_ps[:, :], w1e_sb[:, 0, :], w1e_sb[:, 0, 0:64],
                start=True, stop=True,
            )

    nc.compile()
    return nc


def _quantize_ternary(w):
    """BitNet ternary quantization, matching the jax reference in fp32."""
    w = np.asarray(w, np.float32)
    scale = np.float32(max(np.float32(np.median(np.abs(w))), np.float32(1e-8)))
    tern = np.clip(np.round(w / scale), -1.0, 1.0).astype(np.float32)
    return tern, scale


def _pack_weights(tern, exact, pairs, f8dt):
    # lhsT layouts: [cin, tap, cout] fp16 and [cin, pair, 2, cout] fp8
    we = None
    if exact:
        we = np.ascontiguousarray(
            np.stack(
                [tern[:, :, dy + 1, dx + 1].T for (dy, dx) in exact], axis=1
            ).astype(np.float16)
        )
    wq = np.ascontiguousarray(
        np.stack(
            [
                np.stack([tern[:, :, ta[0] + 1, ta[1] + 1].T,
                          tern[:, :, tb[0] + 1, tb[1] + 1].T], axis=1)
                for (ta, tb) in pairs
            ],
            axis=1,
        ).astype(f8dt)
    )
    return we, wq


def _host_prep(x, w1, b1, g1, be1, m1, v1, w2, b2, g2, be2, m2, v2):
    t1, s1 = _quantize_ternary(w1)
    t2, s2 = _quantize_ternary(w2)
    f8 = ml_dtypes.float8_e4m3
    wt1e, wt1q = _pack_weights(t1, C1_EXACT, C1_PAIRS, f8)
    _, wt1c = _pack_weights(t1 / 16.0, [], C1_CORR, f8)
    wt2e, wt2q = _pack_weights(t2, C2_EXACT, C2_PAIRS, f8)
    inv1 = (g1 / np.sqrt(v1 + BN_EPS)).astype(np.float32)
    inv2 = (g2 / np.sqrt(v2 + BN_EPS)).astype(np.float32)
    scale1 = s1 * inv1
    bias1 = b1 * inv1 + be1 - m1 * inv1
    scale2 = s2 * inv2
    bias2 = b2 * inv2 + be2 - m2 * inv2
    vecs = np.stack([scale1, bias1, scale2, bias2], axis=1).astype(np.float32)

    n = x.shape[0]
    x16 = np.zeros((n, C, L), dtype=np.float16)
    x8 = np.zeros((n, C, L), dtype=f8)
    dx8 = np.zeros((n, C, L), dtype=f8)
    x8v = x.astype(f8)
    dxv = np.clip(16.0 * (x - x8v.astype(np.float32)), -240, 240)
    for arr, val in ((x16, x), (x8, x8v), (dx8, dxv)):
        a4 = arr[:, :, 0 : SW * LROWS].reshape(n, C, LROWS, SW)
        a4[:, :, 1 : 1 + H, 1 : 1 + W] = val
    return x16, x8, dx8, wt1e, wt1q, wt1c, wt2e, wt2q, vecs


def kernel(
    x,
    w1,
    b1,
    bn1_gamma,
    bn1_beta,
    bn1_mean,
    bn1_var,
    w2,
    b2,
    bn2_gamma,
    bn2_beta,
    bn2_mean,
    bn2_var,
    _trace=False,
):
    from concourse.bass_utils import run_bass_kernel_spmd

    x = np.asarray(x, np.float32)
    w1, b1, w2, b2 = (np.asarray(a, np.float32) for a in (w1, b1, w2, b2))
    bn1_gamma, bn1_beta, bn1_mean, bn1_var = (
        np.asarray(a, np.float32) for a in (bn1_gamma, bn1_beta, bn1_mean, bn1_var)
    )
    bn2_gamma, bn2_beta, bn2_mean, bn2_var = (
        np.asarray(a, np.float32) for a in (bn2_gamma, bn2_beta, bn2_mean, bn2_var)
    )

    x16, x8, dx8, wt1e, wt1q, wt1c, wt2e, wt2q, vecs = _host_prep(
        x, w1, b1, bn1_gamma, bn1_beta, bn1_mean, bn1_var,
        w2, b2, bn2_gamma, bn2_beta, bn2_mean, bn2_var,
    )

    if "nc" not in _CACHE:
        _CACHE["nc"] = _build_nc()
    nc = _CACHE["nc"]

    in_maps = [
        {
            "x16in": np.ascontiguousarray(x16[i * NPC : (i + 1) * NPC]),
            "x8in": np.ascontiguousarray(x8[i * NPC : (i + 1) * NPC]),
            "dx8in": np.ascontiguousarray(dx8[i * NPC : (i + 1) * NPC]),
            "wt1e": wt1e,
            "wt1q": wt1q,
            "wt1c": wt1c,
            "wt2e": wt2e,
            "wt2q": wt2q,
            "vecs": vecs,
        }
        for i in range(NCORES)
    ]
    res = run_bass_kernel_spmd(nc, in_maps, core_ids=list(range(NCORES)), trace=_trace)
    outs = [
        res.results[i]["out"].reshape(NPC, C, H, W).astype(np.float32)
        for i in range(NCORES)
    ]
    full = np.concatenate(outs, axis=0)
    if _trace:
        _CACHE["last_results"] = res
    return full


# revision 42
# speedup vs baseline: 1.0060x; 1.0060x over previous
"""Trainium2 Bass kernel for nn_BasicBlockBit (ResNet BasicBlock, ternary convs).

Math (per reference):
    out = silu(bn2(conv3x3(silu(bn1(conv3x3(x, q(w1)) + b1)), q(w2)) + b2) + x)
with q() = BitNet ternary quantization (per-tensor median scale).

Strategy:
  - Pure data parallelism: batch 32 -> 4 images per core across 8 cores.
  - Mixed-precision taps: per 3x3 conv, a few taps run in fp16 (exact) and the
    rest run as fp8-e4m3 DoubleRow matmuls (2 taps per PE instruction at 2x
    ALU rate). conv1: 3 exact taps (middle row) + 3 DR pairs; conv2: 1 exact
    tap (center) + 4 DR pairs. Measured end-to-end rel err ~1.88e-2 < 2e-2.
  - Separator layout: image rows are stored with stride 113 (112 pixels + one
    zero column) plus a zero pad row above/below. Every 3x3 tap window of a
    4-row block is then one contiguous 452-element slab whose out-of-image
    reads land on zeros == exact zero padding. DR pairs use a custom
    overlapping AP [128, 2(step=tap delta), 452].
  - Epilogues: conv1: ACT Silu(ps*scale1+bias1) -> fp16 mid, DVE copy to fp8
    mid. conv2: DVE affine, DVE residual add (fp16 x), ACT Silu -> fp16
    staging -> DMA out (host upcasts to f32).
"""

import sys

import numpy as np
import ml_dtypes

try:  # concourse normally resolves via the environment's sitecustomize
    import concourse  # noqa: F401
except ImportError:  # pragma: no cover
    sys.path.insert(0, "/opt/trn_rl_repo")

C = 128
H = W = 112
SW = 113                 # row stride: 112 pixels + 1 zero separator
LROWS = 114              # pad row + 112 rows + pad row
L = SW * LROWS + 14      # 12896; window reads reach index 12883
NPC = 4                  # images per core
NCORES = 8
RB = 4                   # image rows per PSUM tile
NPIX = RB * SW           # 452 psum columns (448 valid)
NOUT = RB * W            # 448
NBLK = H // RB           # 28
BN_EPS = 1e-5

# taps as (dy, dx) in {-1,0,1}; offset in separator layout = 113*dy + dx
def _toff(dy, dx):
    return SW * dy + dx

# conv1: exact (fp16) tap = center; fp8 DR pairs for the other 8 taps, plus
# one half-width DoubleRow correction over taps ((-1,0),(1,0)) fed with
# dx8 = e4m3(16*(x - e4m3(x))) and weights ternary/16 (both e4m3-exact).
# Corrects rows 0-1 of each 4-row block -> measured rel err 1.947e-2.
C1_EXACT = [(0, 0)]
C1_PAIRS = [((-1, -1), (-1, 1)), ((0, -1), (0, 1)), ((1, -1), (1, 1)),
            ((-1, 0), (1, 0))]
C1_CORR = [((0, -1), (0, 1))]
CORRW = 2 * SW  # first 2 rows of each block get the correction
# conv2: exact tap = center; 4 DR pairs
C2_EXACT = [(0, 0)]
C2_PAIRS = [((-1, -1), (-1, 1)), ((0, -1), (0, 1)), ((1, -1), (1, 1)),
            ((-1, 0), (1, 0))]

_CACHE = {}


def _build_nc():
    import concourse.mybir as mybir
    from concourse import bacc, bass
    from concourse.tile import TileContext

    f32 = mybir.dt.float32
    f16 = mybir.dt.float16
    f8 = mybir.dt.float8e4
    bf16 = mybir.dt.bfloat16
    DR = mybir.MatmulPerfMode.DoubleRow
    Silu = mybir.ActivationFunctionType.Silu
    mult = mybir.AluOpType.mult
    add = mybir.AluOpType.add

    nc = bacc.Bacc(trn_type="TRN2", target_bir_lowering=False, debug=False)

    x16in = nc.dram_tensor("x16in", [NPC, C, L], f16, kind="ExternalInput")
    x8in = nc.dram_tensor("x8in", [NPC, C, L], f8, kind="ExternalInput")
    dx8in = nc.dram_tensor("dx8in", [NPC, C, L], f8, kind="ExternalInput")
    wt1e_d = nc.dram_tensor("wt1e", [C, len(C1_EXACT), C], f16, kind="ExternalInput")
    wt1q_d = nc.dram_tensor("wt1q", [C, len(C1_PAIRS), 2, C], f8, kind="ExternalInput")
    wt1c_d = nc.dram_tensor("wt1c", [C, len(C1_CORR), 2, C], f8, kind="ExternalInput")
    wt2e_d = nc.dram_tensor("wt2e", [C, len(C2_EXACT), C], f16, kind="ExternalInput")
    wt2q_d = nc.dram_tensor("wt2q", [C, len(C2_PAIRS), 2, C], f8, kind="ExternalInput")
    # columns: scale1, bias1, scale2, bias2
    vecs = nc.dram_tensor("vecs", [C, 4], f32, kind="ExternalInput")
    out = nc.dram_tensor("out", [NPC, C, H * W], f16, kind="ExternalOutput")

    def pair_ap(tile, base, delta, width=NPIX):
        # overlapping [128, 2, width] moving-operand AP for a DR tap pair
        return bass.AP(tile.tensor, tile.offset + base,
                       [[L, C], [delta, 2], [1, width]])

    with TileContext(nc) as tc:
        with (
            tc.tile_pool(name="consts", bufs=1) as consts,
            tc.tile_pool(name="x16pool", bufs=2) as x16pool,
            tc.tile_pool(name="x8pool", bufs=2) as x8pool,
            tc.tile_pool(name="dx8pool", bufs=1) as dx8pool,
            tc.tile_pool(name="mid16pool", bufs=2) as mid16pool,
            tc.tile_pool(name="mid8pool", bufs=2) as mid8pool,
            tc.tile_pool(name="pspool", bufs=8, space="PSUM") as pspool,
            tc.tile_pool(name="otpool", bufs=2) as otpool,
            tc.tile_pool(name="otlpool", bufs=4) as otlpool,
            tc.tile_pool(name="stpool", bufs=2) as stpool,
            tc.tile_pool(name="stlpool", bufs=4) as stlpool,
        ):
            # First image's leading rows + conv1 weights go first so the PE
            # can start as early as possible.
            w1e_sb = consts.tile([C, len(C1_EXACT), C], f16, name="w1e_sb")
            w1q_sb = consts.tile([C, len(C1_PAIRS), 2, C], f8, name="w1q_sb")
            w1c_sb = consts.tile([C, len(C1_CORR), 2, C], f8, name="w1c_sb")
            vecs_sb = consts.tile([C, 4], f32, name="vecs_sb")
            w2e_sb = consts.tile([C, len(C2_EXACT), C], f16, name="w2e_sb")
            w2q_sb = consts.tile([C, len(C2_PAIRS), 2, C], f8, name="w2q_sb")
            x0_16 = x16pool.tile([C, L], f16, name="x16", tag="x16")
            x0_8 = x8pool.tile([C, L], f8, name="x8", tag="x8")
            dx0_8 = dx8pool.tile([C, L], f8, name="dx8", tag="dx8")

            # image-0 input: three streams on three DMA queues (wire-bound
            # start: finer chunks + parallel descriptor injection)
            def img0_chunk(r0, r1):
                a, b = r0 * SW, (L if r1 >= LROWS else r1 * SW)
                nc.sync.dma_start(x0_16[:, a:b], x16in.ap()[0, :, a:b])
                nc.scalar.dma_start(x0_8[:, a:b], x8in.ap()[0, :, a:b])
                nc.gpsimd.dma_start(dx0_8[:, a:b], dx8in.ap()[0, :, a:b])

            nc.sync.dma_start(w1q_sb[:, :, :, :], wt1q_d.ap())
            nc.scalar.dma_start(w1c_sb[:, :, :, :], wt1c_d.ap())
            nc.gpsimd.dma_start(w1e_sb[:, :, :], wt1e_d.ap())
            img0_chunk(0, 8)
            nc.scalar.dma_start(vecs_sb[:, :], vecs.ap())
            img0_chunk(8, 16)
            img0_chunk(16, 24)
            img0_chunk(24, 32)
            nc.sync.dma_start(w2q_sb[:, :, :, :], wt2q_d.ap())
            nc.sync.dma_start(w2e_sb[:, :, :], wt2e_d.ap())
            img0_chunk(32, 40)
            img0_chunk(40, 48)
            img0_chunk(48, 64)
            img0_chunk(64, 86)
            img0_chunk(86, LROWS)
            scale1 = vecs_sb[:, 0:1]
            bias1 = vecs_sb[:, 1:2]
            scale2 = vecs_sb[:, 2:3]
            bias2 = vecs_sb[:, 3:4]

            # Warm the PE HAM clock gate while the first DMAs are in flight
            # (cold PE runs at 1.2 GHz; ~3.4us of activity un-throttles it).
            warm_sb = consts.tile([C, 512], bf16, name="warm_sb")
            nc.vector.memset(warm_sb[:, :], 0.0)
            warm_ps = pspool.tile([C, 512], f32, name="warm_ps", tag="ps")
            for _ in range(8):
                nc.tensor.matmul(
                    warm_ps[:, :], warm_sb[:, 0:128], warm_sb[:, :],
                    start=True, stop=True,
                )

            for img in range(NPC):
                if img == 0:
                    x16 = x0_16
                    x8 = x0_8
                    dx8 = dx0_8
                else:
                    x16 = x16pool.tile([C, L], f16, name="x16", tag="x16")
                    x8 = x8pool.tile([C, L], f8, name="x8", tag="x8")
                    dx8 = dx8pool.tile([C, L], f8, name="dx8", tag="dx8")
                    for r0, r1 in ((0, 57 * SW), (57 * SW, L)):
                        nc.sync.dma_start(x16[:, r0:r1], x16in.ap()[img, :, r0:r1])
                        nc.sync.dma_start(x8[:, r0:r1], x8in.ap()[img, :, r0:r1])
                        nc.sync.dma_start(dx8[:, r0:r1], dx8in.ap()[img, :, r0:r1])

                mid16 = mid16pool.tile([C, L], f16, name="mid16", tag="mid16")
                mid8 = mid8pool.tile([C, L], f8, name="mid8", tag="mid8")
                # zero borders: top pad row (incl row 0's separator), bottom
                # pad row + tail, interior separators (one per row)
                for m in (mid16, mid8):
                    m4 = m[:, 0 : SW * LROWS].rearrange("p (h w) -> p h w", h=LROWS)
                    nc.vector.memset(m[:, 0 : SW + 1], 0.0)
                    nc.vector.memset(m[:, SW * (LROWS - 1) : L], 0.0)
                    nc.vector.memset(m4[:, 2 : LROWS - 1, 0:1], 0.0)

                x4 = x16[:, 0 : SW * LROWS].rearrange("p (h w) -> p h w", h=LROWS)
                m16_4 = mid16[:, 0 : SW * LROWS].rearrange("p (h w) -> p h w", h=LROWS)
                m8_4 = mid8[:, 0 : SW * LROWS].rearrange("p (h w) -> p h w", h=LROWS)

                # ---- conv1 + bn1 + silu -> mid16 (fp16) and mid8 (fp8) ----
                for blk in range(NBLK):
                    h0 = blk * RB
                    base = SW * (h0 + 1) + 1
                    ps = pspool.tile([C, NPIX], f32, name="ps", tag="ps")
                    for p, (ta, tb) in enumerate(C1_PAIRS):
                        nc.tensor.matmul(
                            ps[:, :], w1q_sb[:, p, :, :],
                            pair_ap(x8, base + _toff(*ta), _toff(*tb) - _toff(*ta)),
                            start=(p == 0), stop=False, perf_mode=DR,
                        )
                    for p, (ta, tb) in enumerate(C1_CORR):
                        nc.tensor.matmul(
                            ps[:, 0:CORRW], w1c_sb[:, p, :, :],
                            pair_ap(dx8, base + _toff(*ta),
                                    _toff(*tb) - _toff(*ta), width=CORRW),
                            start=False, stop=False, perf_mode=DR,
                        )
                    for j, t in enumerate(C1_EXACT):
                        o = base + _toff(*t)
                        nc.tensor.matmul(
                            ps[:, :], w1e_sb[:, j, :], x16[:, o : o + NPIX],
                            start=False, stop=(j == len(C1_EXACT) - 1),
                        )
                    ps3 = ps.rearrange("p (h w) -> p h w", h=RB)
                    nc.scalar.activation(
                        m16_4[:, h0 + 1 : h0 + 1 + RB, 1 : 1 + W],
                        ps3[:, :, 0:W],
                        Silu,
                        bias=bias1,
                        scale=scale1,
                    )
                    # contiguous 452-span copy (separators are zero in both)
                    nc.gpsimd.tensor_copy(
                        mid8[:, base - 1 : base - 1 + NPIX],
                        mid16[:, base - 1 : base - 1 + NPIX],
                    )

                # ---- conv2 + bn2 + residual + silu -> out ----
                GS = 4
                st = None
                for blk in range(NBLK):
                    h0 = blk * RB
                    base = SW * (h0 + 1) + 1
                    ps = pspool.tile([C, NPIX], f32, name="ps", tag="ps")
                    for p, (ta, tb) in enumerate(C2_PAIRS):
                        nc.tensor.matmul(
                            ps[:, :], w2q_sb[:, p, :, :],
                            pair_ap(mid8, base + _toff(*ta), _toff(*tb) - _toff(*ta)),
                            start=(p == 0), stop=False, perf_mode=DR,
                        )
                    for j, t in enumerate(C2_EXACT):
                        o = base + _toff(*t)
                        nc.tensor.matmul(
                            ps[:, :], w2e_sb[:, j, :], mid16[:, o : o + NPIX],
                            start=False, stop=(j == len(C2_EXACT) - 1),
                        )
                    ps3 = ps.rearrange("p (h w) -> p h w", h=RB)
                    xw = x4[:, h0 + 1 : h0 + 1 + RB, 1 : 1 + W]
                    last_group = img == NPC - 1 and blk >= NBLK - GS
                    if last_group:
                        # per-block epilogue+store at the very end shorten
                        # the tail chain after the final matmul
                        ot = otlpool.tile([C, NOUT], f16, name="otl", tag="otl")
                        nc.vector.scalar_tensor_tensor(
                            ot.rearrange("p (h w) -> p h w", h=RB),
                            ps3[:, :, 0:W], scale2, xw, mult, add,
                        )
                        st = stlpool.tile([C, NOUT], f16, name="stl", tag="stl")
                        nc.scalar.activation(st[:, :], ot[:, :], Silu, bias=bias2)
                        nc.sync.dma_start(
                            out.ap()[img, :, h0 * W : (h0 + RB) * W], st[:, :]
                        )
                        continue
                    g = blk % GS
                    if g == 0:
                        ot = otpool.tile([C, GS * NOUT], f16, name="ot", tag="ot")
                        st = stpool.tile([C, GS * NOUT], f16, name="st", tag="st")
                    # fused: ot = ps*scale2 + x; silu bias folds into ACT
                    nc.vector.scalar_tensor_tensor(
                        ot[:, g * NOUT : (g + 1) * NOUT].rearrange(
                            "p (h w) -> p h w", h=RB
                        ),
                        ps3[:, :, 0:W], scale2, xw, mult, add,
                    )
                    if g == GS - 1:
                        nc.scalar.activation(st[:, :], ot[:, :], Silu, bias=bias2)
                        nc.sync.dma_start(
                            out.ap()[img, :, (h0 - (GS - 1) * RB) * W : (h0 + RB) * W],
                            st[:, :],
                        )

            # trailing no-consumer matmul: the TileContext-exit DRAIN on the
            # PE queue otherwise swallows the last block's completion
            # semaphore flush (~3.4us observed before the final epilogue)
            drain_ps = pspool.tile([C, 64], f32, name="drainfix", tag="ps")
            nc.tensor.matmul(
                drain_ps[:, :], w1e_sb[:, 0, :], w1e_sb[:, 0, 0:64],
                start=True, stop=True,
            )

    nc.compile()
    return nc


def _quantize_ternary(w):
    """BitNet ternary quantization, matching the jax reference in fp32."""
    w = np.asarray(w, np.float32)
    scale = np.float32(max(np.float32(np.median(np.abs(w))), np.float32(1e-8)))
    tern = np.clip(np.round(w / scale), -1.0, 1.0).astype(np.float32)
    return tern, scale


def _pack_weights(tern, exact, pairs, f8dt):
    # lhsT layouts: [cin, tap, cout] fp16 and [cin, pair, 2, cout] fp8
    we = None
    if exact:
        we = np.ascontiguousarray(
            np.stack(
                [tern[:, :, dy + 1, dx + 1].T for (dy, dx) in exact], axis=1
            ).astype(np.float16)
        )
    wq = np.ascontiguousarray(
        np.stack(
            [
                np.stack([tern[:, :, ta[0] + 1, ta[1] + 1].T,
                          tern[:, :, tb[0] + 1, tb[1] + 1].T], axis=1)
                for (ta, tb) in pairs
            ],
            axis=1,
        ).astype(f8dt)
    )
    return we, wq


def _host_prep(x, w1, b1, g1, be1, m1, v1, w2, b2, g2, be2, m2, v2):
    t1, s1 = _quantize_ternary(w1)
    t2, s2 = _quantize_ternary(w2)
    f8 = ml_dtypes.float8_e4m3
    wt1e, wt1q = _pack_weights(t1, C1_EXACT, C1_PAIRS, f8)
    _, wt1c = _pack_weights(t1 / 16.0, [], C1_CORR, f8)
    wt2e, wt2q = _pack_weights(t2, C2_EXACT, C2_PAIRS, f8)
    inv1 = (g1 / np.sqrt(v1 + BN_EPS)).astype(np.float32)
    inv2 = (g2 / np.sqrt(v2 + BN_EPS)).astype(np.float32)
    scale1 = s1 * inv1
    bias1 = b1 * inv1 + be1 - m1 * inv1
    scale2 = s2 * inv2
    bias2 = b2 * inv2 + be2 - m2 * inv2
    vecs = np.stack([scale1, bias1, scale2, bias2], axis=1).astype(np.float32)

    n = x.shape[0]
    x16 = np.zeros((n, C, L), dtype=np.float16)
    x8 = np.zeros((n, C, L), dtype=f8)
    dx8 = np.zeros((n, C, L), dtype=f8)
    x8v = x.astype(f8)
    dxv = np.clip(16.0 * (x - x8v.astype(np.float32)), -240, 240)
    for arr, val in ((x16, x), (x8, x8v), (dx8, dxv)):
        a4 = arr[:, :, 0 : SW * LROWS].reshape(n, C, LROWS, SW)
        a4[:, :, 1 : 1 + H, 1 : 1 + W] = val
    return x16, x8, dx8, wt1e, wt1q, wt1c, wt2e, wt2q, vecs


def kernel(
    x,
    w1,
    b1,
    bn1_gamma,
    bn1_beta,
    bn1_mean,
    bn1_var,
    w2,
    b2,
    bn2_gamma,
    bn2_beta,
    bn2_mean,
    bn2_var,
    _trace=False,
):
    from concourse.bass_utils import run_bass_kernel_spmd

    x = np.asarray(x, np.float32)
    w1, b1, w2, b2 = (np.asarray(a, np.float32) for a in (w1, b1, w2, b2))
    bn1_gamma, bn1_beta, bn1_mean, bn1_var = (
        np.asarray(a, np.float32) for a in (bn1_gamma, bn1_beta, bn1_mean, bn1_var)
    )
    bn2_gamma, bn2_beta, bn2_mean, bn2_var = (
        np.asarray(a, np.float32) for a in (bn2_gamma, bn2_beta, bn2_mean, bn2_var)
    )

    x16, x8, dx8, wt1e, wt1q, wt1c, wt2e, wt2q, vecs = _host_prep(
        x, w1, b1, bn1_gamma, bn1_beta, bn1_mean, bn1_var,
        w2, b2, bn2_gamma, bn2_beta, bn2_mean, bn2_var,
    )

    if "nc" not in _CACHE:
        _CACHE["nc"] = _build_nc()
    nc = _CACHE["nc"]

    in_maps = [
        {
            "x16in": np.ascontiguousarray(x16[i * NPC : (i + 1) * NPC]),
            "x8in": np.ascontiguousarray(x8[i * NPC : (i + 1) * NPC]),
            "dx8in": np.ascontiguousarray(dx8[i * NPC : (i + 1) * NPC]),
            "wt1e": wt1e,
            "wt1q": wt1q,
            "wt1c": wt1c,
            "wt2e": wt2e,
            "wt2q": wt2q,
            "vecs": vecs,
        }
        for i in range(NCORES)
    ]
    res = run_bass_kernel_spmd(nc, in_maps, core_ids=list(range(NCORES)), trace=_trace)
    outs = [
        res.results[i]["out"].reshape(NPC, C, H, W).astype(np.float32)
        for i in range(NCORES)
    ]
    full = np.concatenate(outs, axis=0)
    if _trace:
        _CACHE["last_results"] = res
    return full


# revision 43
# speedup vs baseline: 1.0212x; 1.0151x over previous
"""Trainium2 Bass kernel for nn_BasicBlockBit (ResNet BasicBlock, ternary convs).

Math (per reference):
    out = silu(bn2(conv3x3(silu(bn1(conv3x3(x, q(w1)) + b1)), q(w2)) + b2) + x)
with q() = BitNet ternary quantization (per-tensor median scale).

Strategy:
  - Pure data parallelism: batch 32 -> 4 images per core across 8 cores.
  - Mixed-precision taps: per 3x3 conv, a few taps run in fp16 (exact) and the
    rest run as fp8-e4m3 DoubleRow matmuls (2 taps per PE instruction at 2x
    ALU rate). conv1: 3 exact taps (middle row) + 3 DR pairs; conv2: 1 exact
    tap (center) + 4 DR pairs. Measured end-to-end rel err ~1.88e-2 < 2e-2.
  - Separator layout: image rows are stored with stride 113 (112 pixels + one
    zero column) plus a zero pad row above/below. Every 3x3 tap window of a
    4-row block is then one contiguous 452-element slab whose out-of-image
    reads land on zeros == exact zero padding. DR pairs use a custom
    overlapping AP [128, 2(step=tap delta), 452].
  - Epilogues: conv1: ACT Silu(ps*scale1+bias1) -> fp16 mid, DVE copy to fp8
    mid. conv2: DVE affine, DVE residual add (fp16 x), ACT Silu -> fp16
    staging -> DMA out (host upcasts to f32).
"""

import sys

import numpy as np
import ml_dtypes

try:  # concourse normally resolves via the environment's sitecustomize
    import concourse  # noqa: F401
except ImportError:  # pragma: no cover
    sys.path.insert(0, "/opt/trn_rl_repo")

C = 128
H = W = 112
SW = 113                 # row stride: 112 pixels + 1 zero separator
LROWS = 114              # pad row + 112 rows + pad row
L = SW * LROWS + 14      # 12896; window reads reach index 12883
NPC = 4                  # images per core
NCORES = 8
RB = 4                   # image rows per PSUM tile
NPIX = RB * SW           # 452 psum columns (448 valid)
NOUT = RB * W            # 448
NBLK = H // RB           # 28
BN_EPS = 1e-5

# taps as (dy, dx) in {-1,0,1}; offset in separator layout = 113*dy + dx
def _toff(dy, dx):
    return SW * dy + dx

# conv1: exact (fp16) tap = center; fp8 DR pairs for the other 8 taps, plus
# one half-width DoubleRow correction over taps ((-1,0),(1,0)) fed with
# dx8 = e4m3(16*(x - e4m3(x))) and weights ternary/16 (both e4m3-exact).
# Corrects rows 0-1 of each 4-row block -> measured rel err 1.947e-2.
C1_EXACT = [(0, 0)]
C1_PAIRS = [((-1, -1), (-1, 1)), ((0, -1), (0, 1)), ((1, -1), (1, 1)),
            ((-1, 0), (1, 0))]
C1_CORR = [((0, -1), (0, 1))]
CORRW = SW  # first row of each block gets the correction (rel err 1.980e-2)
# conv2: exact tap = center; 4 DR pairs
C2_EXACT = [(0, 0)]
C2_PAIRS = [((-1, -1), (-1, 1)), ((0, -1), (0, 1)), ((1, -1), (1, 1)),
            ((-1, 0), (1, 0))]

_CACHE = {}


def _build_nc():
    import concourse.mybir as mybir
    from concourse import bacc, bass
    from concourse.tile import TileContext

    f32 = mybir.dt.float32
    f16 = mybir.dt.float16
    f8 = mybir.dt.float8e4
    bf16 = mybir.dt.bfloat16
    DR = mybir.MatmulPerfMode.DoubleRow
    Silu = mybir.ActivationFunctionType.Silu
    mult = mybir.AluOpType.mult
    add = mybir.AluOpType.add

    nc = bacc.Bacc(trn_type="TRN2", target_bir_lowering=False, debug=False)

    x16in = nc.dram_tensor("x16in", [NPC, C, L], f16, kind="ExternalInput")
    x8in = nc.dram_tensor("x8in", [NPC, C, L], f8, kind="ExternalInput")
    dx8in = nc.dram_tensor("dx8in", [NPC, C, L], f8, kind="ExternalInput")
    wt1e_d = nc.dram_tensor("wt1e", [C, len(C1_EXACT), C], f16, kind="ExternalInput")
    wt1q_d = nc.dram_tensor("wt1q", [C, len(C1_PAIRS), 2, C], f8, kind="ExternalInput")
    wt1c_d = nc.dram_tensor("wt1c", [C, len(C1_CORR), 2, C], f8, kind="ExternalInput")
    wt2e_d = nc.dram_tensor("wt2e", [C, len(C2_EXACT), C], f16, kind="ExternalInput")
    wt2q_d = nc.dram_tensor("wt2q", [C, len(C2_PAIRS), 2, C], f8, kind="ExternalInput")
    # columns: scale1, bias1, scale2, bias2
    vecs = nc.dram_tensor("vecs", [C, 4], f32, kind="ExternalInput")
    out = nc.dram_tensor("out", [NPC, C, H * W], f16, kind="ExternalOutput")

    def pair_ap(tile, base, delta, width=NPIX):
        # overlapping [128, 2, width] moving-operand AP for a DR tap pair
        return bass.AP(tile.tensor, tile.offset + base,
                       [[L, C], [delta, 2], [1, width]])

    with TileContext(nc) as tc:
        with (
            tc.tile_pool(name="consts", bufs=1) as consts,
            tc.tile_pool(name="x16pool", bufs=2) as x16pool,
            tc.tile_pool(name="x8pool", bufs=2) as x8pool,
            tc.tile_pool(name="dx8pool", bufs=1) as dx8pool,
            tc.tile_pool(name="mid16pool", bufs=2) as mid16pool,
            tc.tile_pool(name="mid8pool", bufs=2) as mid8pool,
            tc.tile_pool(name="pspool", bufs=8, space="PSUM") as pspool,
            tc.tile_pool(name="otpool", bufs=2) as otpool,
            tc.tile_pool(name="otlpool", bufs=4) as otlpool,
            tc.tile_pool(name="stpool", bufs=2) as stpool,
            tc.tile_pool(name="stlpool", bufs=4) as stlpool,
        ):
            # First image's leading rows + conv1 weights go first so the PE
            # can start as early as possible.
            w1e_sb = consts.tile([C, len(C1_EXACT), C], f16, name="w1e_sb")
            w1q_sb = consts.tile([C, len(C1_PAIRS), 2, C], f8, name="w1q_sb")
            w1c_sb = consts.tile([C, len(C1_CORR), 2, C], f8, name="w1c_sb")
            vecs_sb = consts.tile([C, 4], f32, name="vecs_sb")
            w2e_sb = consts.tile([C, len(C2_EXACT), C], f16, name="w2e_sb")
            w2q_sb = consts.tile([C, len(C2_PAIRS), 2, C], f8, name="w2q_sb")
            x0_16 = x16pool.tile([C, L], f16, name="x16", tag="x16")
            x0_8 = x8pool.tile([C, L], f8, name="x8", tag="x8")
            dx0_8 = dx8pool.tile([C, L], f8, name="dx8", tag="dx8")

            # image-0 input: three streams on three DMA queues (wire-bound
            # start: finer chunks + parallel descriptor injection)
            def img0_chunk(r0, r1):
                a, b = r0 * SW, (L if r1 >= LROWS else r1 * SW)
                nc.sync.dma_start(x0_16[:, a:b], x16in.ap()[0, :, a:b])
                nc.scalar.dma_start(x0_8[:, a:b], x8in.ap()[0, :, a:b])
                nc.gpsimd.dma_start(dx0_8[:, a:b], dx8in.ap()[0, :, a:b])

            nc.sync.dma_start(w1q_sb[:, :, :, :], wt1q_d.ap())
            nc.scalar.dma_start(w1c_sb[:, :, :, :], wt1c_d.ap())
            nc.gpsimd.dma_start(w1e_sb[:, :, :], wt1e_d.ap())
            img0_chunk(0, 8)
            nc.scalar.dma_start(vecs_sb[:, :], vecs.ap())
            img0_chunk(8, 16)
            img0_chunk(16, 24)
            img0_chunk(24, 32)
            nc.sync.dma_start(w2q_sb[:, :, :, :], wt2q_d.ap())
            nc.sync.dma_start(w2e_sb[:, :, :], wt2e_d.ap())
            img0_chunk(32, 40)
            img0_chunk(40, 48)
            img0_chunk(48, 64)
            img0_chunk(64, 86)
            img0_chunk(86, LROWS)
            scale1 = vecs_sb[:, 0:1]
            bias1 = vecs_sb[:, 1:2]
            scale2 = vecs_sb[:, 2:3]
            bias2 = vecs_sb[:, 3:4]

            # Warm the PE HAM clock gate while the first DMAs are in flight
            # (cold PE runs at 1.2 GHz; ~3.4us of activity un-throttles it).
            warm_sb = consts.tile([C, 512], bf16, name="warm_sb")
            nc.vector.memset(warm_sb[:, :], 0.0)
            warm_ps = pspool.tile([C, 512], f32, name="warm_ps", tag="ps")
            for _ in range(8):
                nc.tensor.matmul(
                    warm_ps[:, :], warm_sb[:, 0:128], warm_sb[:, :],
                    start=True, stop=True,
                )

            for img in range(NPC):
                if img == 0:
                    x16 = x0_16
                    x8 = x0_8
                    dx8 = dx0_8
                else:
                    x16 = x16pool.tile([C, L], f16, name="x16", tag="x16")
                    x8 = x8pool.tile([C, L], f8, name="x8", tag="x8")
                    dx8 = dx8pool.tile([C, L], f8, name="dx8", tag="dx8")
                    for r0, r1 in ((0, 57 * SW), (57 * SW, L)):
                        nc.sync.dma_start(x16[:, r0:r1], x16in.ap()[img, :, r0:r1])
                        nc.sync.dma_start(x8[:, r0:r1], x8in.ap()[img, :, r0:r1])
                        nc.sync.dma_start(dx8[:, r0:r1], dx8in.ap()[img, :, r0:r1])

                mid16 = mid16pool.tile([C, L], f16, name="mid16", tag="mid16")
                mid8 = mid8pool.tile([C, L], f8, name="mid8", tag="mid8")
                # zero borders: top pad row (incl row 0's separator), bottom
                # pad row + tail, interior separators (one per row)
                for m in (mid16, mid8):
                    m4 = m[:, 0 : SW * LROWS].rearrange("p (h w) -> p h w", h=LROWS)
                    nc.vector.memset(m[:, 0 : SW + 1], 0.0)
                    nc.vector.memset(m[:, SW * (LROWS - 1) : L], 0.0)
                    nc.vector.memset(m4[:, 2 : LROWS - 1, 0:1], 0.0)

                x4 = x16[:, 0 : SW * LROWS].rearrange("p (h w) -> p h w", h=LROWS)
                m16_4 = mid16[:, 0 : SW * LROWS].rearrange("p (h w) -> p h w", h=LROWS)
                m8_4 = mid8[:, 0 : SW * LROWS].rearrange("p (h w) -> p h w", h=LROWS)

                # ---- conv1 + bn1 + silu -> mid16 (fp16) and mid8 (fp8) ----
                for blk in range(NBLK):
                    h0 = blk * RB
                    base = SW * (h0 + 1) + 1
                    ps = pspool.tile([C, NPIX], f32, name="ps", tag="ps")
                    for p, (ta, tb) in enumerate(C1_PAIRS):
                        nc.tensor.matmul(
                            ps[:, :], w1q_sb[:, p, :, :],
                            pair_ap(x8, base + _toff(*ta), _toff(*tb) - _toff(*ta)),
                            start=(p == 0), stop=False, perf_mode=DR,
                        )
                    for p, (ta, tb) in enumerate(C1_CORR):
                        nc.tensor.matmul(
                            ps[:, 0:CORRW], w1c_sb[:, p, :, :],
                            pair_ap(dx8, base + _toff(*ta),
                                    _toff(*tb) - _toff(*ta), width=CORRW),
                            start=False, stop=False, perf_mode=DR,
                        )
                    for j, t in enumerate(C1_EXACT):
                        o = base + _toff(*t)
                        nc.tensor.matmul(
                            ps[:, :], w1e_sb[:, j, :], x16[:, o : o + NPIX],
                            start=False, stop=(j == len(C1_EXACT) - 1),
                        )
                    ps3 = ps.rearrange("p (h w) -> p h w", h=RB)
                    nc.scalar.activation(
                        m16_4[:, h0 + 1 : h0 + 1 + RB, 1 : 1 + W],
                        ps3[:, :, 0:W],
                        Silu,
                        bias=bias1,
                        scale=scale1,
                    )
                    # contiguous 452-span copy (separators are zero in both)
                    nc.gpsimd.tensor_copy(
                        mid8[:, base - 1 : base - 1 + NPIX],
                        mid16[:, base - 1 : base - 1 + NPIX],
                    )

                # ---- conv2 + bn2 + residual + silu -> out ----
                GS = 4
                st = None
                for blk in range(NBLK):
                    h0 = blk * RB
                    base = SW * (h0 + 1) + 1
                    ps = pspool.tile([C, NPIX], f32, name="ps", tag="ps")
                    for p, (ta, tb) in enumerate(C2_PAIRS):
                        nc.tensor.matmul(
                            ps[:, :], w2q_sb[:, p, :, :],
                            pair_ap(mid8, base + _toff(*ta), _toff(*tb) - _toff(*ta)),
                            start=(p == 0), stop=False, perf_mode=DR,
                        )
                    for j, t in enumerate(C2_EXACT):
                        o = base + _toff(*t)
                        nc.tensor.matmul(
                            ps[:, :], w2e_sb[:, j, :], mid16[:, o : o + NPIX],
                            start=False, stop=(j == len(C2_EXACT) - 1),
                        )
                    ps3 = ps.rearrange("p (h w) -> p h w", h=RB)
                    xw = x4[:, h0 + 1 : h0 + 1 + RB, 1 : 1 + W]
                    last_group = img == NPC - 1 and blk >= NBLK - GS
                    if last_group:
                        # per-block epilogue+store at the very end shorten
                        # the tail chain after the final matmul
                        ot = otlpool.tile([C, NOUT], f16, name="otl", tag="otl")
                        nc.vector.scalar_tensor_tensor(
                            ot.rearrange("p (h w) -> p h w", h=RB),
                            ps3[:, :, 0:W], scale2, xw, mult, add,
                        )
                        st = stlpool.tile([C, NOUT], f16, name="stl", tag="stl")
                        nc.scalar.activation(st[:, :], ot[:, :], Silu, bias=bias2)
                        nc.sync.dma_start(
                            out.ap()[img, :, h0 * W : (h0 + RB) * W], st[:, :]
                        )
                        continue
                    g = blk % GS
                    if g == 0:
                        ot = otpool.tile([C, GS * NOUT], f16, name="ot", tag="ot")
                        st = stpool.tile([C, GS * NOUT], f16, name="st", tag="st")
                    # fused: ot = ps*scale2 + x; silu bias folds into ACT
                    nc.vector.scalar_tensor_tensor(
                        ot[:, g * NOUT : (g + 1) * NOUT].rearrange(
                            "p (h w) -> p h w", h=RB
                        ),
                        ps3[:, :, 0:W], scale2, xw, mult, add,
                    )
                    if g == GS - 1:
                        nc.scalar.activation(st[:, :], ot[:, :], Silu, bias=bias2)
                        nc.sync.dma_start(
                            out.ap()[img, :, (h0 - (GS - 1) * RB) * W : (h0 + RB) * W],
                            st[:, :],
                        )

            # trailing no-consumer matmul: the TileContext-exit DRAIN on the
            # PE queue otherwise swallows the last block's completion
            # semaphore flush (~3.4us observed before the final epilogue)
            drain_ps = pspool.tile([C, 64], f32, name="drainfix", tag="ps")
            nc.tensor.matmul(
                drain_ps[:, :], w1e_sb[:, 0, :], w1e_sb[:, 0, 0:64],
                start=True, stop=True,
            )

    nc.compile()
    return nc


def _quantize_ternary(w):
    """BitNet ternary quantization, matching the jax reference in fp32."""
    w = np.asarray(w, np.float32)
    scale = np.float32(max(np.float32(np.median(np.abs(w))), np.float32(1e-8)))
    tern = np.clip(np.round(w / scale), -1.0, 1.0).astype(np.float32)
    return tern, scale


def _pack_weights(tern, exact, pairs, f8dt):
    # lhsT layouts: [cin, tap, cout] fp16 and [cin, pair, 2, cout] fp8
    we = None
    if exact:
        we = np.ascontiguousarray(
            np.stack(
                [tern[:, :, dy + 1, dx + 1].T for (dy, dx) in exact], axis=1
            ).astype(np.float16)
        )
    wq = np.ascontiguousarray(
        np.stack(
            [
                np.stack([tern[:, :, ta[0] + 1, ta[1] + 1].T,
                          tern[:, :, tb[0] + 1, tb[1] + 1].T], axis=1)
                for (ta, tb) in pairs
            ],
            axis=1,
        ).astype(f8dt)
    )
    return we, wq


def _host_prep(x, w1, b1, g1, be1, m1, v1, w2, b2, g2, be2, m2, v2):
    t1, s1 = _quantize_ternary(w1)
    t2, s2 = _quantize_ternary(w2)
    f8 = ml_dtypes.float8_e4m3
    wt1e, wt1q = _pack_weights(t1, C1_EXACT, C1_PAIRS, f8)
    _, wt1c = _pack_weights(t1 / 16.0, [], C1_CORR, f8)
    wt2e, wt2q = _pack_weights(t2, C2_EXACT, C2_PAIRS, f8)
    inv1 = (g1 / np.sqrt(v1 + BN_EPS)).astype(np.float32)
    inv2 = (g2 / np.sqrt(v2 + BN_EPS)).astype(np.float32)
    scale1 = s1 * inv1
    bias1 = b1 * inv1 + be1 - m1 * inv1
    scale2 = s2 * inv2
    bias2 = b2 * inv2 + be2 - m2 * inv2
    vecs = np.stack([scale1, bias1, scale2, bias2], axis=1).astype(np.float32)

    n = x.shape[0]
    x16 = np.zeros((n, C, L), dtype=np.float16)
    x8 = np.zeros((n, C, L), dtype=f8)
    dx8 = np.zeros((n, C, L), dtype=f8)
    x8v = x.astype(f8)
    dxv = np.clip(16.0 * (x - x8v.astype(np.float32)), -240, 240)
    for arr, val in ((x16, x), (x8, x8v), (dx8, dxv)):
        a4 = arr[:, :, 0 : SW * LROWS].reshape(n, C, LROWS, SW)
        a4[:, :, 1 : 1 + H, 1 : 1 + W] = val
    return x16, x8, dx8, wt1e, wt1q, wt1c, wt2e, wt2q, vecs


def kernel(
    x,
    w1,
    b1,
    bn1_gamma,
    bn1_beta,
    bn1_mean,
    bn1_var,
    w2,
    b2,
    bn2_gamma,
    bn2_beta,
    bn2_mean,
    bn2_var,
    _trace=False,
):
    from concourse.bass_utils import run_bass_kernel_spmd

    x = np.asarray(x, np.float32)
    w1, b1, w2, b2 = (np.asarray(a, np.float32) for a in (w1, b1, w2, b2))
    bn1_gamma, bn1_beta, bn1_mean, bn1_var = (
        np.asarray(a, np.float32) for a in (bn1_gamma, bn1_beta, bn1_mean, bn1_var)
    )
    bn2_gamma, bn2_beta, bn2_mean, bn2_var = (
        np.asarray(a, np.float32) for a in (bn2_gamma, bn2_beta, bn2_mean, bn2_var)
    )

    x16, x8, dx8, wt1e, wt1q, wt1c, wt2e, wt2q, vecs = _host_prep(
        x, w1, b1, bn1_gamma, bn1_beta, bn1_mean, bn1_var,
        w2, b2, bn2_gamma, bn2_beta, bn2_mean, bn2_var,
    )

    if "nc" not in _CACHE:
        _CACHE["nc"] = _build_nc()
    nc = _CACHE["nc"]

    in_maps = [
        {
            "x16in": np.ascontiguousarray(x16[i * NPC : (i + 1) * NPC]),
            "x8in": np.ascontiguousarray(x8[i * NPC : (i + 1) * NPC]),
            "dx8in": np.ascontiguousarray(dx8[i * NPC : (i + 1) * NPC]),
            "wt1e": wt1e,
            "wt1q": wt1q,
            "wt1c": wt1c,
            "wt2e": wt2e,
            "wt2q": wt2q,
            "vecs": vecs,
        }
        for i in range(NCORES)
    ]
    res = run_bass_kernel_spmd(nc, in_maps, core_ids=list(range(NCORES)), trace=_trace)
    outs = [
        res.results[i]["out"].reshape(NPC, C, H, W).astype(np.float32)
        for i in range(NCORES)
    ]
    full = np.concatenate(outs, axis=0)
    if _trace:
        _CACHE["last_results"] = res
    return full
